# revision 38
# baseline (speedup 1.0000x reference)
"""DeepSeek MLA decode attention kernel for 8 Trainium2 NeuronCores.

Sharding: head-parallel projections (16 heads/core), batch-parallel attention
(4 sequences/core, latent KV cache sharded by sequence), head-parallel output
projection.  Cross-core movement: one small AllGather after the low-rank
q_a/kv_a projections, an AllToAll of per-head latent queries into
batch-sharded attention, an AllToAll of attention outputs back to head-sharded
layout.  Final o_proj partial sums are combined on the host.
"""

import numpy as np
import ml_dtypes

BF16 = ml_dtypes.bfloat16

# problem dims (hardcoded per spec)
B, HID, H = 32, 5120, 128
DN, DR, DV = 128, 64, 128
QLR, KVLR = 1536, 512
BLK, NBLK_PER_SEQ = 128, 16
S = BLK * NBLK_PER_SEQ            # 2048
D = KVLR + DR                     # 576
NCORES = 8
HL = H // NCORES                  # 16 local heads
SEQL = B // NCORES                # 4 local seqs
EPS = 1e-6
SCALE = float((DN + DR) ** -0.5)
QKV_SH = QLR // NCORES            # 192
KV_SH = D // NCORES               # 72
AKV = QKV_SH + KV_SH              # 264
NHID = HID // 128                 # 40
NQLR = QLR // 128                 # 12

_prog_cache = {}


def _build(sbn, masked, inj_records, n_rec):
    """Build + compile the 8-core SPMD program.

    sbn: tuple of 4 ints — number of 128-token chunks per local-seq slot.
    masked: apply per-position masks after exp.
    inj_records: tuple of (s_local, p) new-token injection sites (uniform
        across cores), or None for host-side injection fallback.
    n_rec: number of columns in the sel input (>=1).
    """
    from concourse import bass, bacc, tile, mybir, masks

    dt = mybir.dt
    AF = mybir.ActivationFunctionType
    OP = mybir.AluOpType
    AX = mybir.AxisListType

    nc = bacc.Bacc("TRN2", target_bir_lowering=False, debug=False,
                   num_devices=NCORES)

    def din(name, shape, dtype=dt.bfloat16):
        return nc.dram_tensor(name, shape, dtype, kind="ExternalInput").ap()

    hiddenT = din("hiddenT", [NHID, 128, B])
    wqakv = din("wqakv", [NHID, 128, AKV])
    wqb = din("wqb", [NQLR, 128, HL * (DN + DR)])
    wuk = din("wuk", [HL, 4, 128, 128])
    wuv = din("wuv", [HL, 4, 128, 128])
    wo = din("wo", [HL, 128, HID])
    ckvT = din("ckvT", [SEQL, 4, 4, 128, 512])
    cpeT = din("cpeT", [SEQL, DR, S])
    keysN = din("keysN", [SEQL, NBLK_PER_SEQ, 128, KVLR])
    cosrep = din("cosrep", [B, HL * (DR // 2)], dt.float32)
    sinrep = din("sinrep", [B, HL * (DR // 2)], dt.float32)
    cospe = din("cospe", [B, DR // 2], dt.float32)
    sinpe = din("sinpe", [B, DR // 2], dt.float32)
    kvw = din("kvw", [B, KVLR], dt.float32)
    sel = din("sel", [B, n_rec])
    maskt = din("maskt", [SEQL, NBLK_PER_SEQ, 128]) if masked else None

    out = nc.dram_tensor("out", [B, HID], dt.float32,
                         kind="ExternalOutput").ap()
    kvdbg = nc.dram_tensor("kvdbg", [1, KVLR + 16], dt.float32,
                           kind="ExternalOutput").ap()

    RG = [list(range(NCORES))]

    with tile.TileContext(nc) as tc:
        with tc.tile_pool(name="const", bufs=1) as constp, \
             tc.tile_pool(name="dram", bufs=1, space="DRAM") as dram, \
             tc.tile_pool(name="persist", bufs=1) as pp:
            ident = constp.tile([128, 128], dt.bfloat16)
            masks.make_identity(nc, ident[:])
            ones_bf = constp.tile([128, 1], dt.bfloat16)
            nc.vector.memset(ones_bf[:], 1.0)
            ones_f32 = constp.tile([1, 128], dt.float32)
            nc.vector.memset(ones_f32[:], 1.0)
            epsc = constp.tile([128, 1], dt.float32)
            nc.vector.memset(epsc[:], EPS)

            ag1_in = dram.tile([B, AKV], dt.float32)
            ag1_out = dram.tile([NCORES, B, AKV], dt.float32)
            a2aq_in = [dram.tile([NCORES, 128, 2, 5, HL], dt.bfloat16,
                                 name=f"a2aq_in{g}", tag=f"a2aq_in{g}")
                       for g in range(2)]
            a2aq_out = [dram.tile([NCORES, 128, 2, 5, HL], dt.bfloat16,
                                  name=f"a2aq_out{g}", tag=f"a2aq_out{g}")
                        for g in range(2)]
            a2ac_in = [dram.tile([NCORES, 4, 128, HL], dt.bfloat16,
                                 name=f"a2ac_in{s_}", tag=f"a2ac_in{s_}")
                       for s_ in range(SEQL)]
            a2ac_out = [dram.tile([NCORES, 4, 128, HL], dt.bfloat16,
                                  name=f"a2ac_out{s_}", tag=f"a2ac_out{s_}")
                        for s_ in range(SEQL)]

            asmt = [pp.tile([128, SEQL * NCORES * HL], dt.bfloat16,
                            name=f"asm{cc}", tag=f"asm{cc}")
                    for cc in range(5)]
            kv_bf = pp.tile([B, D], dt.bfloat16, name="kv_bf")
            nc.vector.memset(asmt[4][:], 0.0)
            warm_sb = pp.tile([128, 512], dt.bfloat16, name="warm_sb")
            nc.vector.memset(warm_sb[:], 0.001)
            war_in = dram.tile([1, 8], dt.float32)
            war_out = dram.tile([1, 8], dt.float32)
            war2_in = dram.tile([1, 8], dt.float32)
            war2_out = dram.tile([1, 8], dt.float32)
            nc.sync.dma_start(war2_in[:], ones_f32[:1, :8])
            nc.sync.dma_start(war_in[:], ones_f32[:1, :8])
            war3_in = dram.tile([1, 8], dt.float32)
            war3_out = dram.tile([1, 8], dt.float32)
            nc.sync.dma_start(war3_in[:], ones_f32[:1, :8])

            def warm_pe(wp, n_mm, tagn, nbufs=1):
                for i in range(n_mm):
                    wps = wp.tile([128, 512], dt.float32, tag=tagn,
                                  name="wps", bufs=nbufs)
                    nc.tensor.matmul(wps[:], warm_sb[:, :128], warm_sb[:],
                                     start=True, stop=True)

            with tc.tile_pool(name="wres", bufs=1) as wres:
                # resident weights for phase A2 (loaded during launch skew)
                wqb_res = wres.tile([128, NQLR, HL * (DN + DR)], dt.bfloat16,
                                    name="wqb_res")
                wuk_res = wres.tile([128, HL, 4, 128], dt.bfloat16,
                                    name="wuk_res")

                # ---------------- Phase A1: q_a + kv_a projection ------------
                with tc.tile_pool(name="pa", bufs=3) as pa, \
                     tc.tile_pool(name="pa_ps", bufs=1, space="PSUM") as pap:
                    acc = pap.tile([B, AKV], dt.float32)
                    for g in range(NHID // 4):
                        hidt = pa.tile([128, 4, B], dt.bfloat16, tag="hidt")
                        nc.sync.dma_start(
                            hidt[:],
                            hiddenT[4 * g:4 * g + 4].transpose([1, 0, 2]))
                        wt = pa.tile([128, 4, AKV], dt.bfloat16, tag="wt")
                        nc.gpsimd.dma_start(
                            wt[:],
                            wqakv[4 * g:4 * g + 4].transpose([1, 0, 2]))
                        for j in range(4):
                            i = 4 * g + j
                            nc.tensor.matmul(acc[:], hidt[:, j], wt[:, j],
                                             start=(i == 0),
                                             stop=(i == NHID - 1))
                    qakv_sb = pa.tile([B, AKV], dt.float32, tag="qakv")
                    nc.vector.tensor_copy(qakv_sb[:], acc[:])
                    nc.sync.dma_start(ag1_in[:], qakv_sb[:])

                with tc.tile_pool(name="wmp", bufs=1,
                                  space="PSUM") as wmp:
                    warm_pe(wmp, 80, "wm0")

                nc.gpsimd.collective_compute(
                    "AllGather", OP.bypass, replica_groups=RG,
                    ins=[ag1_in.opt()], outs=[ag1_out.opt()])

                for i in range(NQLR):
                    _eng = [nc.sync, nc.scalar, nc.gpsimd][i % 3]
                    _eng.dma_start(wqb_res[:, i], wqb[i])
                for h in range(HL):
                    _eng = [nc.gpsimd, nc.scalar, nc.sync][h % 3]
                    _eng.dma_start(wuk_res[:, h], wuk[h].transpose([1, 0, 2]))

                # ------------- Phase A2: norms, q_b, rope, q_lat -------------
                with tc.tile_pool(name="pb", bufs=1) as pb, \
                     tc.tile_pool(name="pb_ps", bufs=2, space="PSUM") as pbp, \
                     tc.tile_pool(name="pb_ps2", bufs=1,
                                  space="PSUM") as pbp2:
                    q_a = pb.tile([B, QLR], dt.float32)
                    nc.sync.dma_start(
                        q_a[:].rearrange("b (r d) -> b r d", r=NCORES),
                        ag1_out[:, :, 0:QKV_SH].transpose([1, 0, 2]))
                    kvn = pb.tile([B, D], dt.float32)
                    nc.sync.dma_start(
                        kvn[:].rearrange("b (r d) -> b r d", r=NCORES),
                        ag1_out[:, :, QKV_SH:AKV].transpose([1, 0, 2]))

                    # q rmsnorm (weight folded into Wq_b on host)
                    sq = pb.tile([B, QLR], dt.float32)
                    nc.vector.tensor_tensor(sq[:], q_a[:], q_a[:], op=OP.mult)
                    ssq = pb.tile([B, 1], dt.float32)
                    nc.vector.tensor_reduce(ssq[:], sq[:], axis=AX.X,
                                            op=OP.add)
                    rms = pb.tile([B, 1], dt.float32)
                    nc.scalar.activation(rms[:], ssq[:], AF.Sqrt,
                                         bias=epsc[:B, :1], scale=1.0 / QLR)
                    nc.sync.dma_start(war_in[:1, :1], ssq[:1, :1])
                    qinv = pb.tile([B, 1], dt.float32)
                    nc.vector.reciprocal(qinv[:], rms[:])
                    q_nrm = pb.tile([B, QLR], dt.bfloat16)
                    nc.vector.tensor_scalar_mul(q_nrm[:], q_a[:], qinv[:, :1])

                    # kv rmsnorm * weight + rope(k_pe)
                    ksq = pb.tile([B, KVLR], dt.float32)
                    nc.vector.tensor_tensor(ksq[:], kvn[:, :KVLR],
                                            kvn[:, :KVLR], op=OP.mult)
                    kssq = pb.tile([B, 1], dt.float32)
                    nc.vector.tensor_reduce(kssq[:], ksq[:], axis=AX.X,
                                            op=OP.add)
                    krms = pb.tile([B, 1], dt.float32)
                    nc.scalar.activation(krms[:], kssq[:], AF.Sqrt,
                                         bias=epsc[:B, :1], scale=1.0 / KVLR)
                    kinv = pb.tile([B, 1], dt.float32)
                    nc.vector.reciprocal(kinv[:], krms[:])
                    kvlat = pb.tile([B, KVLR], dt.float32)
                    nc.vector.tensor_scalar_mul(kvlat[:], kvn[:, :KVLR],
                                                kinv[:, :1])
                    kvw_sb = pb.tile([B, KVLR], dt.float32)
                    nc.scalar.dma_start(kvw_sb[:], kvw[:])
                    nc.vector.tensor_tensor(kv_bf[:, :KVLR], kvlat[:],
                                            kvw_sb[:], op=OP.mult)

                    cospe_sb = pb.tile([B, DR // 2], dt.float32)
                    nc.scalar.dma_start(cospe_sb[:], cospe[:])
                    sinpe_sb = pb.tile([B, DR // 2], dt.float32)
                    nc.scalar.dma_start(sinpe_sb[:], sinpe[:])
                    px1 = kvn[:, KVLR:D:2]
                    px2 = kvn[:, KVLR + 1:D:2]
                    t1 = pb.tile([B, DR // 2], dt.float32, tag="t1")
                    t2 = pb.tile([B, DR // 2], dt.float32, tag="t2")
                    nc.vector.tensor_tensor(t1[:], px1, cospe_sb[:],
                                            op=OP.mult)
                    nc.vector.tensor_tensor(t2[:], px2, sinpe_sb[:],
                                            op=OP.mult)
                    nc.vector.tensor_tensor(kv_bf[:, KVLR:D:2], t1[:], t2[:],
                                            op=OP.subtract)
                    nc.vector.tensor_tensor(t1[:], px2, cospe_sb[:],
                                            op=OP.mult)
                    nc.vector.tensor_tensor(t2[:], px1, sinpe_sb[:],
                                            op=OP.mult)
                    nc.vector.tensor_tensor(kv_bf[:, KVLR + 1:D:2], t1[:],
                                            t2[:], op=OP.add)

                    # new-token injection into staged caches (uniform mode)
                    if inj_records is not None:
                        sel_sb = pb.tile([B, n_rec], dt.bfloat16)
                        nc.scalar.dma_start(sel_sb[:], sel[:])
                        for j, (s, p) in enumerate(inj_records):
                            selc = sel_sb[:, j:j + 1]
                            row_ps = pbp.tile([1, KVLR], dt.float32,
                                              tag="inj", name="row_ps", bufs=1)
                            nc.tensor.matmul(row_ps[:], selc, kv_bf[:, :KVLR],
                                             start=True, stop=True)
                            row_bf = pb.tile([1, KVLR], dt.bfloat16,
                                             tag="rowbf")
                            nc.vector.tensor_copy(row_bf[:], row_ps[:])
                            nc.scalar.dma_start(
                                keysN[s, p // 128, p % 128:p % 128 + 1, :],
                                row_bf[:])
                            for cc in range(4):
                                cps = pbp.tile([128, 1], dt.float32,
                                               tag="inj", name="cps", bufs=1)
                                nc.tensor.matmul(
                                    cps[:],
                                    kv_bf[:, cc * 128:(cc + 1) * 128],
                                    selc, start=True, stop=True)
                                cbf = pb.tile([128, 1], dt.bfloat16,
                                              tag="cbf")
                                nc.vector.tensor_copy(cbf[:], cps[:])
                                nc.scalar.dma_start(
                                    ckvT[s, p // 512, cc, :,
                                         p % 512:p % 512 + 1], cbf[:])
                            pps = pbp.tile([DR, 1], dt.float32, tag="inj",
                                           name="pps", bufs=1)
                            nc.tensor.matmul(pps[:], kv_bf[:, KVLR:D], selc,
                                             start=True, stop=True)
                            pbf = pb.tile([DR, 1], dt.bfloat16, tag="pbf")
                            nc.vector.tensor_copy(pbf[:], pps[:])
                            nc.scalar.dma_start(cpeT[s, :, p:p + 1], pbf[:])

                    # kvdbg keeps the kv path live regardless of mode
                    dbg_ps = pbp.tile([1, KVLR], dt.float32, tag="inj",
                                      name="dbg_ps", bufs=1)
                    nc.tensor.matmul(dbg_ps[:], ones_bf[:B, :],
                                     kv_bf[:, :KVLR], start=True, stop=True)
                    dbg_sb = pb.tile([1, KVLR], dt.float32, tag="dbgs")
                    nc.vector.tensor_copy(dbg_sb[:], dbg_ps[:])
                    nc.scalar.dma_start(kvdbg[:, :KVLR], dbg_sb[:])

                    nc.gpsimd.collective_compute(
                        "AllReduce", OP.add, replica_groups=RG,
                        ins=[war_in.opt()], outs=[war_out.opt()])
                    nc.sync.dma_start(kvdbg[:, KVLR:KVLR + 8], war_out[:])
                    # transpose q_nrm -> qT chunks [128d, 32b]
                    qT = pb.tile([128, NQLR * B], dt.bfloat16)
                    for i in range(NQLR):
                        tp = pbp.tile([128, B], dt.bfloat16, tag="ptp",
                                      name="tp", bufs=4)
                        nc.tensor.transpose(tp[:],
                                            q_nrm[:, i * 128:(i + 1) * 128],
                                            ident[:B, :B])
                        nc.vector.tensor_copy(qT[:, i * B:(i + 1) * B], tp[:])

                    # q_b matmul -> q_full [32, 3072] fp32 (weights resident)
                    q_full = pb.tile([B, HL * (DN + DR)], dt.float32)
                    for half in range(2):
                        qb_ps = [pbp2.tile([B, 512], dt.float32,
                                           tag=f"qb{j}", name=f"qb{j}",
                                           bufs=1)
                                 for j in range(3)]
                        for i in range(NQLR):
                            for j in range(3):
                                n = half * 3 + j
                                nc.tensor.matmul(
                                    qb_ps[j][:], qT[:, i * B:(i + 1) * B],
                                    wqb_res[:, i, n * 512:(n + 1) * 512],
                                    start=(i == 0), stop=(i == NQLR - 1))
                        for j in range(3):
                            n = half * 3 + j
                            nc.vector.tensor_copy(
                                q_full[:, n * 512:(n + 1) * 512], qb_ps[j][:])

                    nc.sync.dma_start(war3_in[:1, :1], q_full[:1, :1])
                    nc.gpsimd.collective_compute(
                        "AllReduce", OP.add, replica_groups=RG,
                        ins=[war3_in.opt()], outs=[war3_out.opt()])
                    nc.sync.dma_start(kvdbg[:, KVLR + 8:KVLR + 12],
                                      war3_out[:1, :4])
                    # rope on q_pe columns (in place on q_full)
                    cosr_sb = pb.tile([B, HL * (DR // 2)], dt.float32)
                    nc.scalar.dma_start(cosr_sb[:], cosrep[:])
                    sinr_sb = pb.tile([B, HL * (DR // 2)], dt.float32)
                    nc.scalar.dma_start(sinr_sb[:], sinrep[:])
                    qv = q_full[:].rearrange("b (h c) -> b h c", h=HL)
                    qx1 = qv[:, :, DN:DN + DR:2]
                    qx2 = qv[:, :, DN + 1:DN + DR:2]
                    cv = cosr_sb[:].rearrange("b (h c) -> b h c", h=HL)
                    sv = sinr_sb[:].rearrange("b (h c) -> b h c", h=HL)
                    r1 = pb.tile([B, HL * (DR // 2)], dt.float32, tag="r1")
                    r2 = pb.tile([B, HL * (DR // 2)], dt.float32, tag="r2")
                    r3 = pb.tile([B, HL * (DR // 2)], dt.float32, tag="r3")
                    r1v = r1[:].rearrange("b (h c) -> b h c", h=HL)
                    r2v = r2[:].rearrange("b (h c) -> b h c", h=HL)
                    r3v = r3[:].rearrange("b (h c) -> b h c", h=HL)
                    nc.vector.tensor_tensor(r1v, qx1, cv, op=OP.mult)
                    nc.vector.tensor_tensor(r2v, qx2, sv, op=OP.mult)
                    nc.vector.tensor_tensor(r3v, r1v, r2v, op=OP.subtract)
                    nc.vector.tensor_tensor(r1v, qx2, cv, op=OP.mult)
                    nc.vector.tensor_tensor(r2v, qx1, sv, op=OP.mult)
                    nc.vector.tensor_copy(qx1, r3v)
                    nc.vector.tensor_tensor(qx2, r1v, r2v, op=OP.add)

                    q_bf = pb.tile([B, HL * (DN + DR)], dt.bfloat16)
                    nc.vector.tensor_copy(q_bf[:], q_full[:])

                    # per-head transposes first, then dense q_lat MMs
                    qnT_all = pb.tile([128, HL * B], dt.bfloat16)
                    for h in range(HL):
                        tpn = pbp.tile([128, B], dt.bfloat16, tag="ptp",
                                       name="tpn", bufs=4)
                        nc.tensor.transpose(tpn[:],
                                            q_bf[:, h * 192:h * 192 + DN],
                                            ident[:B, :B])
                        nc.vector.tensor_copy(qnT_all[:, h * B:(h + 1) * B],
                                              tpn[:])
                        tpp = pbp.tile([DR, B], dt.bfloat16, tag="ptp",
                                       name="tpp", bufs=4)
                        nc.tensor.transpose(
                            tpp[:], q_bf[:, h * 192 + DN:h * 192 + 192],
                            ident[:B, :B])
                        nc.vector.tensor_copy(asmt[4][:DR, h * B:(h + 1) * B],
                                              tpp[:])
                    for h in range(HL):
                        for cc in range(4):
                            ql = pbp.tile([128, B], dt.float32, tag="ptp",
                                          name="ql", bufs=4)
                            nc.tensor.matmul(ql[:], wuk_res[:, h, cc],
                                             qnT_all[:, h * B:(h + 1) * B],
                                             start=True, stop=True)
                            nc.vector.tensor_copy(
                                asmt[cc][:, h * B:(h + 1) * B], ql[:])

            # ship q to the cores owning each sequence (one A2A per seq)
            for s_ in range(SEQL):
                qpiece = pp.tile([128, NCORES * 5 * HL], dt.bfloat16,
                                 name=f"qpiece{s_}", tag=f"qpiece{s_}")
                pv = qpiece[:].rearrange("p (d c h) -> p d c h",
                                         d=NCORES, c=5)
                for cc in range(5):
                    src = asmt[cc][:].rearrange(
                        "p (h d b) -> p h d b", h=HL, d=NCORES)[:, :, :, s_]
                    nc.vector.tensor_copy(pv[:, :, cc, :],
                                          src.transpose([0, 2, 1]))
                g, o = divmod(s_, 2)
                _eng = [nc.scalar, nc.sync, nc.gpsimd, nc.scalar][s_]
                _eng.dma_start(
                    a2aq_in[g][:, :, o:o + 1].transpose(
                        [1, 0, 2, 3, 4]).rearrange(
                        "p d o c h -> p d (o c h)"),
                    qpiece[:].rearrange("p (d x) -> p d x", d=NCORES))
                if s_ % 2 == 1:
                    nc.gpsimd.collective_compute(
                        "AllToAll", OP.bypass, replica_groups=RG,
                        ins=[a2aq_in[g].opt()], outs=[a2aq_out[g].opt()])

            # ---------------- Phase B + C pools (C first for prefetch) ------
            with tc.tile_pool(name="oc", bufs=2) as oc, \
                 tc.tile_pool(name="att", bufs=3) as ab:
              with tc.tile_pool(name="att_ps", bufs=2, space="PSUM") as aps, \
                   tc.tile_pool(name="att_ps1", bufs=1, space="PSUM") as aps1:
                wot_tiles = []
                for n in range(HID // 512):
                    wot = oc.tile([128, HL, 512], dt.bfloat16, tag="wot",
                                  bufs=6, name="wot")
                    nc.gpsimd.dma_start(
                        wot[:],
                        wo[:, :, n * 512:(n + 1) * 512].transpose([1, 0, 2]))
                    wot_tiles.append(wot)

                for s in range(SEQL):
                    rxq_t = ab.tile([128, NCORES, 5 * HL], dt.bfloat16,
                                    tag="rxq", bufs=2, name="rxq_t")
                    for r_ in range(NCORES):
                        nc.sync.dma_start(
                            rxq_t[:, r_],
                            a2aq_out[s // 2][r_, :, s % 2:s % 2 + 1]
                            .rearrange("p o c h -> p (o c h)"))
                    rxv = rxq_t[:].rearrange("p r (c h) -> p r c h", c=5)
                    rxk = ab.tile([128, 5, H], dt.bfloat16, tag="rxk",
                                  bufs=2, name="rxk")
                    rkv = rxk[:].rearrange("p c (r h) -> p c r h", r=NCORES)
                    for cc in range(5):
                        nc.vector.tensor_copy(rkv[:, cc], rxv[:, :, cc, :])
                    ctx_ps = aps1.tile([128, 512], dt.float32, tag="ctx",
                                       name="ctx_ps", bufs=2)
                    dent = ab.tile([128, 4], dt.float32, tag="dent")
                    nblk = (sbn[s] + 3) // 4
                    lastc = sbn[s] - 1
                    for blk in range(nblk):
                        nch = min(4, sbn[s] - 4 * blk)
                        bsl = slice(blk * 512, blk * 512 + nch * 128)
                        ktp = ab.tile([128, 4, 512], dt.bfloat16, tag="ktp",
                                      bufs=4, name="ktp")
                        nc.sync.dma_start(
                            ktp[:, :, :nch * 128],
                            ckvT[s, blk, :, :, :nch * 128].transpose(
                                [1, 0, 2]))
                        ptp2 = ab.tile([DR, 512], dt.bfloat16, tag="ptp2",
                                       bufs=3, name="ptp2")
                        nc.sync.dma_start(ptp2[:, :nch * 128],
                                          cpeT[s, :, bsl])
                        sc_ps = aps.tile([128, 512], dt.float32, tag="sc",
                                         name="sc_ps", bufs=3)
                        for cc in range(4):
                            nc.tensor.matmul(sc_ps[:, :nch * 128],
                                             rxk[:, cc],
                                             ktp[:, cc, :nch * 128],
                                             start=(cc == 0), stop=False)
                        nc.tensor.matmul(sc_ps[:, :nch * 128],
                                         rxk[:DR, 4],
                                         ptp2[:, :nch * 128],
                                         start=False, stop=True)
                        attn_nat = ab.tile([128, 512], dt.bfloat16,
                                           tag="attn_nat", bufs=3)
                        if masked:
                            mrow = ab.tile([1, 512], dt.bfloat16, tag="mrow",
                                           bufs=2)
                            nc.sync.dma_start(
                                mrow[:, :nch * 128],
                                maskt[s].rearrange(
                                    "c p -> (c p)")[bsl].rearrange(
                                    "x -> 1 x"))
                            mb_ps = aps.tile([128, 512], dt.bfloat16,
                                             tag="mb", name="mb_ps", bufs=1)
                            nc.tensor.matmul(mb_ps[:, :nch * 128],
                                             ones_bf[:1, :],
                                             mrow[:, :nch * 128],
                                             start=True, stop=True)
                            mbs = ab.tile([128, 512], dt.bfloat16,
                                          tag="mbs", bufs=2)
                            nc.vector.tensor_copy(mbs[:, :nch * 128],
                                                  mb_ps[:, :nch * 128])
                            nc.scalar.activation(
                                attn_nat[:, :nch * 128],
                                sc_ps[:, :nch * 128], AF.Exp)
                            nc.vector.tensor_tensor(
                                attn_nat[:, :nch * 128],
                                attn_nat[:, :nch * 128],
                                mbs[:, :nch * 128], op=OP.mult)
                            nc.vector.tensor_reduce(
                                dent[:, blk:blk + 1],
                                attn_nat[:, :nch * 128], axis=AX.X,
                                op=OP.add)
                        else:
                            nc.scalar.activation(
                                attn_nat[:, :nch * 128],
                                sc_ps[:, :nch * 128], AF.Exp,
                                accum_out=dent[:, blk:blk + 1])
                        for j in range(nch):
                            ch = 4 * blk + j
                            atT_ps = aps.tile([128, H], dt.bfloat16,
                                              tag="ctf", name="atT_ps",
                                              bufs=2 if masked else 3)
                            nc.tensor.transpose(
                                atT_ps[:],
                                attn_nat[:, j * 128:(j + 1) * 128],
                                ident[:])
                            atT = ab.tile([128, H], dt.bfloat16, tag="atT",
                                          bufs=4)
                            nc.vector.tensor_copy(atT[:], atT_ps[:])
                            knp = ab.tile([128, KVLR], dt.bfloat16,
                                          tag="knp", bufs=4, name="knp")
                            nc.sync.dma_start(knp[:], keysN[s, ch])
                            nc.tensor.matmul(ctx_ps[:], atT[:], knp[:],
                                             start=(ch == 0),
                                             stop=(ch == lastc))
                    den4 = ab.tile([128, 1], dt.float32, tag="den4")
                    nc.vector.tensor_reduce(den4[:], dent[:, :nblk],
                                            axis=AX.X, op=OP.add)
                    inv_col = ab.tile([128, 1], dt.float32, tag="inv_col")
                    nc.vector.reciprocal(inv_col[:], den4[:])
                    if s == 0:
                        nc.sync.dma_start(war2_in[:1, :1], inv_col[:1, :1])
                        nc.gpsimd.collective_compute(
                            "AllReduce", OP.add, replica_groups=RG,
                            ins=[war2_in.opt()], outs=[war2_out.opt()])
                        nc.sync.dma_start(kvdbg[:, KVLR + 12:KVLR + 16],
                                          war2_out[:1, :4])
                    cfn = ab.tile([128, 512], dt.bfloat16, tag="cfn")
                    nc.vector.tensor_scalar_mul(cfn[:], ctx_ps[:],
                                                inv_col[:, :1])
                    for ks in range(4):
                        ctf_ps = aps.tile([128, H], dt.bfloat16, tag="ctf",
                                          name="ctf_ps",
                                          bufs=2 if masked else 3)
                        nc.tensor.transpose(
                            ctf_ps[:], cfn[:, ks * 128:(ks + 1) * 128],
                            ident[:])
                        cf = ab.tile([128, H], dt.bfloat16, tag="cf", bufs=2)
                        nc.vector.tensor_copy(cf[:], ctf_ps[:])
                        _eng = [nc.scalar, nc.sync, nc.gpsimd,
                                nc.scalar][ks]
                        _eng.dma_start(
                            a2ac_in[s][:, ks].transpose([1, 0, 2]),
                            cf[:].rearrange("p (d h) -> p d h", d=NCORES))
                    nc.gpsimd.collective_compute(
                        "AllToAll", OP.bypass, replica_groups=RG,
                        ins=[a2ac_in[s].opt()], outs=[a2ac_out[s].opt()])
              with tc.tile_pool(name="oc_ps", bufs=2, space="PSUM") as ocp:
                rxc = []
                for ks in range(4):
                    t = oc.tile([128, NCORES * SEQL * HL], dt.bfloat16,
                                tag=f"rxc{ks}", bufs=1, name=f"rxc{ks}")
                    for s_ in range(SEQL):
                        nc.scalar.dma_start(
                            t[:].rearrange("p (r s h) -> p r s h",
                                           r=NCORES, s=SEQL)[:, :, s_, :],
                            a2ac_out[s_][:, ks].transpose([1, 0, 2]))
                    rxc.append(t)
                oT = oc.tile([128, HL * B], dt.bfloat16)
                for h in range(HL):
                    op_ps = ocp.tile([128, B], dt.float32, tag="op")
                    wuvt = oc.tile([128, 4, 128], dt.bfloat16, tag="wuvt",
                                   bufs=3)
                    nc.scalar.dma_start(wuvt[:], wuv[h].transpose([1, 0, 2]))
                    for ks in range(4):
                        rhs = rxc[ks][:].rearrange(
                            "p (r s h) -> p r s h", r=NCORES,
                            s=SEQL)[:, :, :, h]
                        nc.tensor.matmul(op_ps[:], wuvt[:, ks], rhs,
                                         start=(ks == 0), stop=(ks == 3))
                    nc.vector.tensor_copy(oT[:, h * B:(h + 1) * B], op_ps[:])
                for n in range(HID // 512):
                    o_ps = ocp.tile([B, 512], dt.float32, tag="ops")
                    for h in range(HL):
                        nc.tensor.matmul(o_ps[:], oT[:, h * B:(h + 1) * B],
                                         wot_tiles[n][:, h], start=(h == 0),
                                         stop=(h == HL - 1))
                    o_sb = oc.tile([B, 512], dt.float32, tag="osb", bufs=2)
                    nc.vector.tensor_copy(o_sb[:], o_ps[:])
                    nc.scalar.dma_start(out[:, n * 512:(n + 1) * 512],
                                        o_sb[:])

    nc.compile()
    return nc


def _stage(inputs, sbn, masked, inj_by_core, n_rec, host_inject):
    """Build per-core input maps (numpy, host-side)."""
    hs = inputs["hidden_states"].astype(np.float32)
    cos = inputs["cos"].astype(np.float32)
    sin = inputs["sin"].astype(np.float32)
    Wq_a = inputs["Wq_a"].astype(np.float32)
    q_a_ln_w = inputs["q_a_ln_w"].astype(np.float32)
    Wq_b = inputs["Wq_b"].astype(np.float32)
    Wkv_a = inputs["Wkv_a"].astype(np.float32)
    kv_a_ln_w = inputs["kv_a_ln_w"].astype(np.float32)
    W_UK_T = inputs["W_UK_T"].astype(np.float32)
    W_UV = inputs["W_UV"].astype(np.float32)
    Wo = inputs["Wo"].astype(np.float32)
    cache_kv = np.asarray(inputs["cache_kv"])
    cache_pe = np.asarray(inputs["cache_pe"])
    block_table = np.asarray(inputs["block_table"])
    seq_lens = np.asarray(inputs["seq_lens"])

    ckv_flat = cache_kv.reshape(-1, KVLR)
    cpe_flat = cache_pe.reshape(-1, DR)
    if host_inject is not None:
        ckv_flat = ckv_flat.copy()
        cpe_flat = cpe_flat.copy()
        kv_lat_h, k_pe_h = host_inject
        slot_mapping = np.asarray(inputs["slot_mapping"])
        for b in range(B):
            ckv_flat[slot_mapping[b]] = kv_lat_h[b]
            cpe_flat[slot_mapping[b]] = k_pe_h[b]

    # scale + ln_w folded into Wq_b
    Wq_b_f = (Wq_b * q_a_ln_w[:, None]) * SCALE
    hiddenT = np.ascontiguousarray(hs.T).reshape(NHID, 128, B).astype(BF16)
    cosrep = np.tile(cos, (1, HL)).astype(np.float32)
    sinrep = np.tile(sin, (1, HL)).astype(np.float32)
    kvw = np.broadcast_to(kv_a_ln_w, (B, KVLR)).astype(np.float32)

    in_maps = []
    for c in range(NCORES):
        m = {}
        m["hiddenT"] = hiddenT
        wq_sh = Wq_a[:, c * QKV_SH:(c + 1) * QKV_SH]
        wkv_sh = Wkv_a[:, c * KV_SH:(c + 1) * KV_SH]
        m["wqakv"] = np.concatenate([wq_sh, wkv_sh], axis=1) \
            .reshape(NHID, 128, AKV).astype(BF16)
        hsl = slice(c * HL, (c + 1) * HL)
        m["wqb"] = Wq_b_f[:, c * HL * 192:(c + 1) * HL * 192] \
            .reshape(NQLR, 128, HL * 192).astype(BF16)
        # wuk[h][ksub][dn][k]: W_UK_T[h] is [DN, KVLR]; ksub splits k
        m["wuk"] = np.ascontiguousarray(
            W_UK_T[hsl].reshape(HL, DN, 4, 128).transpose(0, 2, 1, 3)
        ).astype(BF16)
        m["wuv"] = np.ascontiguousarray(
            W_UV[hsl].reshape(HL, 4, 128, DV)).astype(BF16)
        m["wo"] = np.ascontiguousarray(
            Wo.reshape(H, DV, HID)[hsl]).astype(BF16)

        ckvT = np.empty((SEQL, 4, 4, 128, 512), dtype=BF16)
        cpeT = np.empty((SEQL, DR, S), dtype=BF16)
        keysN = np.empty((SEQL, NBLK_PER_SEQ, 128, KVLR), dtype=BF16)
        for sl_ in range(SEQL):
            g = c * SEQL + sl_
            rows = (block_table[g][:, None] * BLK +
                    np.arange(BLK)[None, :]).reshape(-1)
            gk = ckv_flat[rows]                      # [S, KVLR]
            gp = cpe_flat[rows]                      # [S, DR]
            # [4cc,128dd,4blk,512] -> [4blk,4cc,128dd,512]
            ckvT[sl_] = gk.T.reshape(4, 128, 4, 512).transpose(
                2, 0, 1, 3).astype(BF16)
            cpeT[sl_] = gp.T.astype(BF16)
            keysN[sl_] = gk.reshape(NBLK_PER_SEQ, 128, KVLR).astype(BF16)
        m["ckvT"] = ckvT
        m["cpeT"] = cpeT
        m["keysN"] = keysN
        m["cosrep"] = cosrep
        m["sinrep"] = sinrep
        m["cospe"] = cos
        m["sinpe"] = sin
        m["kvw"] = kvw
        selm = np.zeros((B, n_rec), dtype=BF16)
        if inj_by_core is not None:
            for j, (_s, _p, b) in enumerate(inj_by_core[c]):
                selm[b, j] = 1.0
        m["sel"] = selm
        if masked:
            mk = np.zeros((SEQL, NBLK_PER_SEQ, 128), dtype=BF16)
            for sl_ in range(SEQL):
                L = int(seq_lens[c * SEQL + sl_])
                flat = mk[sl_].reshape(-1)
                flat[:L] = 1.0
            m["maskt"] = mk
        in_maps.append(m)
    return in_maps


def _analyze(inputs):
    """Derive program-shape metadata from the int inputs."""
    seq_lens = np.asarray(inputs["seq_lens"])
    block_table = np.asarray(inputs["block_table"])
    slot_mapping = np.asarray(inputs["slot_mapping"])

    sbn = []
    for sl_ in range(SEQL):
        m = 1
        for c in range(NCORES):
            L = int(seq_lens[c * SEQL + sl_])
            m = max(m, (max(L, 1) + BLK - 1) // BLK)
        sbn.append(m)
    masked = any(
        int(seq_lens[c * SEQL + sl_]) != sbn[sl_] * BLK
        for c in range(NCORES) for sl_ in range(SEQL))

    # injection sites: last write wins per (seq, pos)
    site = {}
    for b in range(B):
        slot = int(slot_mapping[b])
        blk, off = divmod(slot, BLK)
        hits = np.argwhere(block_table == blk)
        for s, j in hits:
            p = int(j) * BLK + int(off)
            if p < int(seq_lens[s]):
                site[(int(s), p)] = b
    by_core = [[] for _ in range(NCORES)]
    for (s, p), b in sorted(site.items()):
        by_core[s // SEQL].append((s % SEQL, p, b))
    pats = [tuple((s, p) for s, p, _b in lst) for lst in by_core]
    uniform = all(p == pats[0] for p in pats)
    if uniform:
        inj_records = pats[0]
        n_rec = max(1, len(inj_records))
        return tuple(sbn), masked, inj_records, [
            lst for lst in by_core], n_rec, False
    return tuple(sbn), masked, None, None, 1, True


def _host_kv(inputs):
    """Host fp32 computation of the new token's kv_lat / k_pe (fallback)."""
    hs = inputs["hidden_states"].astype(np.float32)
    kv = hs @ inputs["Wkv_a"].astype(np.float32)
    kv_lat = kv[:, :KVLR]
    var = (kv_lat * kv_lat).mean(axis=-1, keepdims=True)
    kv_lat = kv_lat / np.sqrt(var + EPS) * \
        inputs["kv_a_ln_w"].astype(np.float32)
    k_pe = kv[:, KVLR:]
    cos = inputs["cos"].astype(np.float32)
    sin = inputs["sin"].astype(np.float32)
    x1 = k_pe[:, 0::2]
    x2 = k_pe[:, 1::2]
    o = np.empty_like(k_pe)
    o[:, 0::2] = x1 * cos - x2 * sin
    o[:, 1::2] = x2 * cos + x1 * sin
    return kv_lat, o


def kernel(**inputs):
    from concourse.bass_utils import run_bass_kernel_spmd

    sbn, masked, inj_records, inj_by_core, n_rec, fallback = _analyze(inputs)
    key = (sbn, masked, inj_records, n_rec)
    if key not in _prog_cache:
        _prog_cache[key] = _build(sbn, masked, inj_records, n_rec)
    nc = _prog_cache[key]

    host_inject = _host_kv(inputs) if fallback else None
    in_maps = _stage(inputs, sbn, masked, inj_by_core, n_rec, host_inject)
    res = run_bass_kernel_spmd(nc, in_maps, core_ids=list(range(NCORES)))
    out = np.zeros((B, HID), dtype=np.float64)
    for r in res.results:
        out += r["out"].astype(np.float64)
    return out.astype(np.float32)


# revision 39
# speedup vs baseline: 1.0801x; 1.0801x over previous
"""DeepSeek MLA decode attention kernel for 8 Trainium2 NeuronCores.

Sharding: head-parallel projections (16 heads/core), batch-parallel attention
(4 sequences/core, latent KV cache sharded by sequence), head-parallel output
projection.  Cross-core movement: one small AllGather after the low-rank
q_a/kv_a projections, an AllToAll of per-head latent queries into
batch-sharded attention, an AllToAll of attention outputs back to head-sharded
layout.  Final o_proj partial sums are combined on the host.
"""

import numpy as np
import ml_dtypes

BF16 = ml_dtypes.bfloat16

# problem dims (hardcoded per spec)
B, HID, H = 32, 5120, 128
DN, DR, DV = 128, 64, 128
QLR, KVLR = 1536, 512
BLK, NBLK_PER_SEQ = 128, 16
S = BLK * NBLK_PER_SEQ            # 2048
D = KVLR + DR                     # 576
NCORES = 8
HL = H // NCORES                  # 16 local heads
SEQL = B // NCORES                # 4 local seqs
EPS = 1e-6
SCALE = float((DN + DR) ** -0.5)
QKV_SH = QLR // NCORES            # 192
KV_SH = D // NCORES               # 72
AKV = QKV_SH + KV_SH              # 264
NHID = HID // 128                 # 40
NQLR = QLR // 128                 # 12

_prog_cache = {}


def _build(sbn, masked, inj_records, n_rec):
    """Build + compile the 8-core SPMD program.

    sbn: tuple of 4 ints — number of 128-token chunks per local-seq slot.
    masked: apply per-position masks after exp.
    inj_records: tuple of (s_local, p) new-token injection sites (uniform
        across cores), or None for host-side injection fallback.
    n_rec: number of columns in the sel input (>=1).
    """
    from concourse import bass, bacc, tile, mybir, masks

    dt = mybir.dt
    AF = mybir.ActivationFunctionType
    OP = mybir.AluOpType
    AX = mybir.AxisListType

    nc = bacc.Bacc("TRN2", target_bir_lowering=False, debug=False,
                   num_devices=NCORES)

    def din(name, shape, dtype=dt.bfloat16):
        return nc.dram_tensor(name, shape, dtype, kind="ExternalInput").ap()

    hiddenT = din("hiddenT", [NHID, 128, B])
    wqakv = din("wqakv", [NHID, 128, AKV])
    wqb = din("wqb", [NQLR, 128, HL * (DN + DR)])
    wuk = din("wuk", [HL, 4, 128, 128])
    wuv = din("wuv", [HL, 4, 128, 128])
    wo = din("wo", [HL, 128, HID])
    ckvT = din("ckvT", [SEQL, 4, 4, 128, 512])
    cpeT = din("cpeT", [SEQL, DR, S])
    keysN = din("keysN", [SEQL, NBLK_PER_SEQ, 128, KVLR])
    cosrep = din("cosrep", [B, HL * (DR // 2)], dt.float32)
    sinrep = din("sinrep", [B, HL * (DR // 2)], dt.float32)
    cospe = din("cospe", [B, DR // 2], dt.float32)
    sinpe = din("sinpe", [B, DR // 2], dt.float32)
    kvw = din("kvw", [B, KVLR], dt.float32)
    sel = din("sel", [B, n_rec])
    maskt = din("maskt", [SEQL, NBLK_PER_SEQ, 128]) if masked else None

    out = nc.dram_tensor("out", [B, HID], dt.float32,
                         kind="ExternalOutput").ap()
    kvdbg = nc.dram_tensor("kvdbg", [1, KVLR + 16], dt.float32,
                           kind="ExternalOutput").ap()

    RG = [list(range(NCORES))]

    with tile.TileContext(nc) as tc:
        with tc.tile_pool(name="const", bufs=1) as constp, \
             tc.tile_pool(name="dram", bufs=1, space="DRAM") as dram, \
             tc.tile_pool(name="persist", bufs=1) as pp:
            ident = constp.tile([128, 128], dt.bfloat16)
            masks.make_identity(nc, ident[:])
            ones_bf = constp.tile([128, 1], dt.bfloat16)
            nc.vector.memset(ones_bf[:], 1.0)
            ones_f32 = constp.tile([1, 128], dt.float32)
            nc.vector.memset(ones_f32[:], 1.0)
            epsc = constp.tile([128, 1], dt.float32)
            nc.vector.memset(epsc[:], EPS)

            ag1_in = dram.tile([B, AKV], dt.float32)
            ag1_out = dram.tile([NCORES, B, AKV], dt.float32)
            a2aq_in = dram.tile([NCORES, 128, SEQL, 5, HL], dt.bfloat16)
            a2aq_out = dram.tile([NCORES, 128, SEQL, 5, HL], dt.bfloat16)
            a2ac_in = [dram.tile([NCORES, 4, 128, HL], dt.bfloat16,
                                 name=f"a2ac_in{s_}", tag=f"a2ac_in{s_}")
                       for s_ in range(SEQL)]
            a2ac_out = [dram.tile([NCORES, 4, 128, HL], dt.bfloat16,
                                  name=f"a2ac_out{s_}", tag=f"a2ac_out{s_}")
                        for s_ in range(SEQL)]

            asmt = [pp.tile([128, SEQL * NCORES * HL], dt.bfloat16,
                            name=f"asm{cc}", tag=f"asm{cc}")
                    for cc in range(5)]
            kv_bf = pp.tile([B, D], dt.bfloat16, name="kv_bf")
            nc.vector.memset(asmt[4][:], 0.0)
            warm_sb = pp.tile([128, 512], dt.bfloat16, name="warm_sb")
            nc.vector.memset(warm_sb[:], 0.001)
            war_in = dram.tile([1, 8], dt.float32)
            war_out = dram.tile([1, 8], dt.float32)
            war2_in = dram.tile([1, 8], dt.float32)
            war2_out = dram.tile([1, 8], dt.float32)
            nc.sync.dma_start(war2_in[:], ones_f32[:1, :8])
            nc.sync.dma_start(war_in[:], ones_f32[:1, :8])


            def warm_pe(wp, n_mm, tagn, nbufs=1):
                for i in range(n_mm):
                    wps = wp.tile([128, 512], dt.float32, tag=tagn,
                                  name="wps", bufs=nbufs)
                    nc.tensor.matmul(wps[:], warm_sb[:, :128], warm_sb[:],
                                     start=True, stop=True)

            with tc.tile_pool(name="wres", bufs=1) as wres:
                # resident weights for phase A2 (loaded during launch skew)
                wqb_res = wres.tile([128, NQLR, HL * (DN + DR)], dt.bfloat16,
                                    name="wqb_res")
                wuk_res = wres.tile([128, HL, 4, 128], dt.bfloat16,
                                    name="wuk_res")

                # ---------------- Phase A1: q_a + kv_a projection ------------
                with tc.tile_pool(name="pa", bufs=3) as pa, \
                     tc.tile_pool(name="pa_ps", bufs=1, space="PSUM") as pap:
                    acc = pap.tile([B, AKV], dt.float32)
                    for g in range(NHID // 4):
                        hidt = pa.tile([128, 4, B], dt.bfloat16, tag="hidt")
                        nc.sync.dma_start(
                            hidt[:],
                            hiddenT[4 * g:4 * g + 4].transpose([1, 0, 2]))
                        wt = pa.tile([128, 4, AKV], dt.bfloat16, tag="wt")
                        nc.gpsimd.dma_start(
                            wt[:],
                            wqakv[4 * g:4 * g + 4].transpose([1, 0, 2]))
                        for j in range(4):
                            i = 4 * g + j
                            nc.tensor.matmul(acc[:], hidt[:, j], wt[:, j],
                                             start=(i == 0),
                                             stop=(i == NHID - 1))
                    qakv_sb = pa.tile([B, AKV], dt.float32, tag="qakv")
                    nc.vector.tensor_copy(qakv_sb[:], acc[:])
                    nc.sync.dma_start(ag1_in[:], qakv_sb[:])

                with tc.tile_pool(name="wmp", bufs=1,
                                  space="PSUM") as wmp:
                    warm_pe(wmp, 80, "wm0")

                nc.gpsimd.collective_compute(
                    "AllGather", OP.bypass, replica_groups=RG,
                    ins=[ag1_in.opt()], outs=[ag1_out.opt()])

                for i in range(NQLR):
                    _eng = [nc.sync, nc.scalar, nc.gpsimd][i % 3]
                    _eng.dma_start(wqb_res[:, i], wqb[i])
                for h in range(HL):
                    _eng = [nc.gpsimd, nc.scalar, nc.sync][h % 3]
                    _eng.dma_start(wuk_res[:, h], wuk[h].transpose([1, 0, 2]))

                # ------------- Phase A2: norms, q_b, rope, q_lat -------------
                with tc.tile_pool(name="pb", bufs=1) as pb, \
                     tc.tile_pool(name="pb_ps", bufs=2, space="PSUM") as pbp, \
                     tc.tile_pool(name="pb_ps2", bufs=1,
                                  space="PSUM") as pbp2:
                    q_a = pb.tile([B, QLR], dt.float32)
                    nc.sync.dma_start(
                        q_a[:].rearrange("b (r d) -> b r d", r=NCORES),
                        ag1_out[:, :, 0:QKV_SH].transpose([1, 0, 2]))
                    kvn = pb.tile([B, D], dt.float32)
                    nc.sync.dma_start(
                        kvn[:].rearrange("b (r d) -> b r d", r=NCORES),
                        ag1_out[:, :, QKV_SH:AKV].transpose([1, 0, 2]))

                    # q rmsnorm (weight folded into Wq_b on host)
                    sq = pb.tile([B, QLR], dt.float32)
                    nc.vector.tensor_tensor(sq[:], q_a[:], q_a[:], op=OP.mult)
                    ssq = pb.tile([B, 1], dt.float32)
                    nc.vector.tensor_reduce(ssq[:], sq[:], axis=AX.X,
                                            op=OP.add)
                    rms = pb.tile([B, 1], dt.float32)
                    nc.scalar.activation(rms[:], ssq[:], AF.Sqrt,
                                         bias=epsc[:B, :1], scale=1.0 / QLR)
                    nc.sync.dma_start(war_in[:1, :1], ssq[:1, :1])
                    qinv = pb.tile([B, 1], dt.float32)
                    nc.vector.reciprocal(qinv[:], rms[:])
                    q_nrm = pb.tile([B, QLR], dt.bfloat16)
                    nc.vector.tensor_scalar_mul(q_nrm[:], q_a[:], qinv[:, :1])

                    # kv rmsnorm * weight + rope(k_pe)
                    ksq = pb.tile([B, KVLR], dt.float32)
                    nc.vector.tensor_tensor(ksq[:], kvn[:, :KVLR],
                                            kvn[:, :KVLR], op=OP.mult)
                    kssq = pb.tile([B, 1], dt.float32)
                    nc.vector.tensor_reduce(kssq[:], ksq[:], axis=AX.X,
                                            op=OP.add)
                    krms = pb.tile([B, 1], dt.float32)
                    nc.scalar.activation(krms[:], kssq[:], AF.Sqrt,
                                         bias=epsc[:B, :1], scale=1.0 / KVLR)
                    kinv = pb.tile([B, 1], dt.float32)
                    nc.vector.reciprocal(kinv[:], krms[:])
                    kvlat = pb.tile([B, KVLR], dt.float32)
                    nc.vector.tensor_scalar_mul(kvlat[:], kvn[:, :KVLR],
                                                kinv[:, :1])
                    kvw_sb = pb.tile([B, KVLR], dt.float32)
                    nc.scalar.dma_start(kvw_sb[:], kvw[:])
                    nc.vector.tensor_tensor(kv_bf[:, :KVLR], kvlat[:],
                                            kvw_sb[:], op=OP.mult)

                    cospe_sb = pb.tile([B, DR // 2], dt.float32)
                    nc.scalar.dma_start(cospe_sb[:], cospe[:])
                    sinpe_sb = pb.tile([B, DR // 2], dt.float32)
                    nc.scalar.dma_start(sinpe_sb[:], sinpe[:])
                    px1 = kvn[:, KVLR:D:2]
                    px2 = kvn[:, KVLR + 1:D:2]
                    t1 = pb.tile([B, DR // 2], dt.float32, tag="t1")
                    t2 = pb.tile([B, DR // 2], dt.float32, tag="t2")
                    nc.vector.tensor_tensor(t1[:], px1, cospe_sb[:],
                                            op=OP.mult)
                    nc.vector.tensor_tensor(t2[:], px2, sinpe_sb[:],
                                            op=OP.mult)
                    nc.vector.tensor_tensor(kv_bf[:, KVLR:D:2], t1[:], t2[:],
                                            op=OP.subtract)
                    nc.vector.tensor_tensor(t1[:], px2, cospe_sb[:],
                                            op=OP.mult)
                    nc.vector.tensor_tensor(t2[:], px1, sinpe_sb[:],
                                            op=OP.mult)
                    nc.vector.tensor_tensor(kv_bf[:, KVLR + 1:D:2], t1[:],
                                            t2[:], op=OP.add)

                    # new-token injection into staged caches (uniform mode)
                    if inj_records is not None:
                        sel_sb = pb.tile([B, n_rec], dt.bfloat16)
                        nc.scalar.dma_start(sel_sb[:], sel[:])
                        for j, (s, p) in enumerate(inj_records):
                            selc = sel_sb[:, j:j + 1]
                            row_ps = pbp.tile([1, KVLR], dt.float32,
                                              tag="inj", name="row_ps", bufs=1)
                            nc.tensor.matmul(row_ps[:], selc, kv_bf[:, :KVLR],
                                             start=True, stop=True)
                            row_bf = pb.tile([1, KVLR], dt.bfloat16,
                                             tag="rowbf")
                            nc.vector.tensor_copy(row_bf[:], row_ps[:])
                            nc.scalar.dma_start(
                                keysN[s, p // 128, p % 128:p % 128 + 1, :],
                                row_bf[:])
                            for cc in range(4):
                                cps = pbp.tile([128, 1], dt.float32,
                                               tag="inj", name="cps", bufs=1)
                                nc.tensor.matmul(
                                    cps[:],
                                    kv_bf[:, cc * 128:(cc + 1) * 128],
                                    selc, start=True, stop=True)
                                cbf = pb.tile([128, 1], dt.bfloat16,
                                              tag="cbf")
                                nc.vector.tensor_copy(cbf[:], cps[:])
                                nc.scalar.dma_start(
                                    ckvT[s, p // 512, cc, :,
                                         p % 512:p % 512 + 1], cbf[:])
                            pps = pbp.tile([DR, 1], dt.float32, tag="inj",
                                           name="pps", bufs=1)
                            nc.tensor.matmul(pps[:], kv_bf[:, KVLR:D], selc,
                                             start=True, stop=True)
                            pbf = pb.tile([DR, 1], dt.bfloat16, tag="pbf")
                            nc.vector.tensor_copy(pbf[:], pps[:])
                            nc.scalar.dma_start(cpeT[s, :, p:p + 1], pbf[:])

                    # kvdbg keeps the kv path live regardless of mode
                    dbg_ps = pbp.tile([1, KVLR], dt.float32, tag="inj",
                                      name="dbg_ps", bufs=1)
                    nc.tensor.matmul(dbg_ps[:], ones_bf[:B, :],
                                     kv_bf[:, :KVLR], start=True, stop=True)
                    dbg_sb = pb.tile([1, KVLR], dt.float32, tag="dbgs")
                    nc.vector.tensor_copy(dbg_sb[:], dbg_ps[:])
                    nc.scalar.dma_start(kvdbg[:, :KVLR], dbg_sb[:])

                    nc.gpsimd.collective_compute(
                        "AllReduce", OP.add, replica_groups=RG,
                        ins=[war_in.opt()], outs=[war_out.opt()])
                    nc.sync.dma_start(kvdbg[:, KVLR:KVLR + 8], war_out[:])
                    # transpose q_nrm -> qT chunks [128d, 32b]
                    qT = pb.tile([128, NQLR * B], dt.bfloat16)
                    for i in range(NQLR):
                        tp = pbp.tile([128, B], dt.bfloat16, tag="ptp",
                                      name="tp", bufs=4)
                        nc.tensor.transpose(tp[:],
                                            q_nrm[:, i * 128:(i + 1) * 128],
                                            ident[:B, :B])
                        nc.vector.tensor_copy(qT[:, i * B:(i + 1) * B], tp[:])

                    # q_b matmul -> q_full [32, 3072] fp32 (weights resident)
                    q_full = pb.tile([B, HL * (DN + DR)], dt.float32)
                    for half in range(2):
                        qb_ps = [pbp2.tile([B, 512], dt.float32,
                                           tag=f"qb{j}", name=f"qb{j}",
                                           bufs=1)
                                 for j in range(3)]
                        for i in range(NQLR):
                            for j in range(3):
                                n = half * 3 + j
                                nc.tensor.matmul(
                                    qb_ps[j][:], qT[:, i * B:(i + 1) * B],
                                    wqb_res[:, i, n * 512:(n + 1) * 512],
                                    start=(i == 0), stop=(i == NQLR - 1))
                        for j in range(3):
                            n = half * 3 + j
                            nc.vector.tensor_copy(
                                q_full[:, n * 512:(n + 1) * 512], qb_ps[j][:])

                    # rope on q_pe columns (in place on q_full)
                    cosr_sb = pb.tile([B, HL * (DR // 2)], dt.float32)
                    nc.scalar.dma_start(cosr_sb[:], cosrep[:])
                    sinr_sb = pb.tile([B, HL * (DR // 2)], dt.float32)
                    nc.scalar.dma_start(sinr_sb[:], sinrep[:])
                    qv = q_full[:].rearrange("b (h c) -> b h c", h=HL)
                    qx1 = qv[:, :, DN:DN + DR:2]
                    qx2 = qv[:, :, DN + 1:DN + DR:2]
                    cv = cosr_sb[:].rearrange("b (h c) -> b h c", h=HL)
                    sv = sinr_sb[:].rearrange("b (h c) -> b h c", h=HL)
                    r1 = pb.tile([B, HL * (DR // 2)], dt.float32, tag="r1")
                    r2 = pb.tile([B, HL * (DR // 2)], dt.float32, tag="r2")
                    r3 = pb.tile([B, HL * (DR // 2)], dt.float32, tag="r3")
                    r1v = r1[:].rearrange("b (h c) -> b h c", h=HL)
                    r2v = r2[:].rearrange("b (h c) -> b h c", h=HL)
                    r3v = r3[:].rearrange("b (h c) -> b h c", h=HL)
                    nc.vector.tensor_tensor(r1v, qx1, cv, op=OP.mult)
                    nc.vector.tensor_tensor(r2v, qx2, sv, op=OP.mult)
                    nc.vector.tensor_tensor(r3v, r1v, r2v, op=OP.subtract)
                    nc.vector.tensor_tensor(r1v, qx2, cv, op=OP.mult)
                    nc.vector.tensor_tensor(r2v, qx1, sv, op=OP.mult)
                    nc.vector.tensor_copy(qx1, r3v)
                    nc.vector.tensor_tensor(qx2, r1v, r2v, op=OP.add)

                    q_bf = pb.tile([B, HL * (DN + DR)], dt.bfloat16)
                    nc.vector.tensor_copy(q_bf[:], q_full[:])

                    # per-head transposes first, then dense q_lat MMs
                    qnT_all = pb.tile([128, HL * B], dt.bfloat16)
                    for h in range(HL):
                        tpn = pbp.tile([128, B], dt.bfloat16, tag="ptp",
                                       name="tpn", bufs=4)
                        nc.tensor.transpose(tpn[:],
                                            q_bf[:, h * 192:h * 192 + DN],
                                            ident[:B, :B])
                        nc.vector.tensor_copy(qnT_all[:, h * B:(h + 1) * B],
                                              tpn[:])
                        tpp = pbp.tile([DR, B], dt.bfloat16, tag="ptp",
                                       name="tpp", bufs=4)
                        nc.tensor.transpose(
                            tpp[:], q_bf[:, h * 192 + DN:h * 192 + 192],
                            ident[:B, :B])
                        nc.vector.tensor_copy(asmt[4][:DR, h * B:(h + 1) * B],
                                              tpp[:])
                    for h in range(HL):
                        for cc in range(4):
                            ql = pbp.tile([128, B], dt.float32, tag="ptp",
                                          name="ql", bufs=4)
                            nc.tensor.matmul(ql[:], wuk_res[:, h, cc],
                                             qnT_all[:, h * B:(h + 1) * B],
                                             start=True, stop=True)
                            nc.vector.tensor_copy(
                                asmt[cc][:, h * B:(h + 1) * B], ql[:])

            # ship q to the cores owning each sequence (one A2A per seq)
            for s_ in range(SEQL):
                qpiece = pp.tile([128, NCORES * 5 * HL], dt.bfloat16,
                                 name=f"qpiece{s_}", tag=f"qpiece{s_}")
                pv = qpiece[:].rearrange("p (d c h) -> p d c h",
                                         d=NCORES, c=5)
                for cc in range(5):
                    src = asmt[cc][:].rearrange(
                        "p (h d b) -> p h d b", h=HL, d=NCORES)[:, :, :, s_]
                    nc.vector.tensor_copy(pv[:, :, cc, :],
                                          src.transpose([0, 2, 1]))
                _eng = [nc.scalar, nc.sync, nc.gpsimd, nc.scalar][s_]
                _eng.dma_start(
                    a2aq_in[:, :, s_:s_ + 1].transpose(
                        [1, 0, 2, 3, 4]).rearrange(
                        "p d o c h -> p d (o c h)"),
                    qpiece[:].rearrange("p (d x) -> p d x", d=NCORES))
            nc.gpsimd.collective_compute(
                "AllToAll", OP.bypass, replica_groups=RG,
                ins=[a2aq_in.opt()], outs=[a2aq_out.opt()])

            # ---------------- Phase B + C pools (C first for prefetch) ------
            with tc.tile_pool(name="oc", bufs=2) as oc, \
                 tc.tile_pool(name="att", bufs=3) as ab:
              with tc.tile_pool(name="att_ps", bufs=2, space="PSUM") as aps, \
                   tc.tile_pool(name="att_ps1", bufs=1, space="PSUM") as aps1:
                wot_tiles = []
                for n in range(HID // 512):
                    wot = oc.tile([128, HL, 512], dt.bfloat16, tag="wot",
                                  bufs=6, name="wot")
                    nc.gpsimd.dma_start(
                        wot[:],
                        wo[:, :, n * 512:(n + 1) * 512].transpose([1, 0, 2]))
                    wot_tiles.append(wot)

                for s in range(SEQL):
                    rxq_t = ab.tile([128, NCORES, 5 * HL], dt.bfloat16,
                                    tag="rxq", bufs=2, name="rxq_t")
                    for r_ in range(NCORES):
                        nc.sync.dma_start(
                            rxq_t[:, r_],
                            a2aq_out[r_, :, s:s + 1].rearrange(
                                "p o c h -> p (o c h)"))
                    rxv = rxq_t[:].rearrange("p r (c h) -> p r c h", c=5)
                    rxk = ab.tile([128, 5, H], dt.bfloat16, tag="rxk",
                                  bufs=2, name="rxk")
                    rkv = rxk[:].rearrange("p c (r h) -> p c r h", r=NCORES)
                    for cc in range(5):
                        nc.vector.tensor_copy(rkv[:, cc], rxv[:, :, cc, :])
                    ctx_ps = aps1.tile([128, 512], dt.float32, tag="ctx",
                                       name="ctx_ps", bufs=2)
                    dent = ab.tile([128, 4], dt.float32, tag="dent")
                    nblk = (sbn[s] + 3) // 4
                    lastc = sbn[s] - 1
                    for blk in range(nblk):
                        nch = min(4, sbn[s] - 4 * blk)
                        bsl = slice(blk * 512, blk * 512 + nch * 128)
                        ktp = ab.tile([128, 4, 512], dt.bfloat16, tag="ktp",
                                      bufs=4, name="ktp")
                        nc.sync.dma_start(
                            ktp[:, :, :nch * 128],
                            ckvT[s, blk, :, :, :nch * 128].transpose(
                                [1, 0, 2]))
                        ptp2 = ab.tile([DR, 512], dt.bfloat16, tag="ptp2",
                                       bufs=3, name="ptp2")
                        nc.sync.dma_start(ptp2[:, :nch * 128],
                                          cpeT[s, :, bsl])
                        sc_ps = aps.tile([128, 512], dt.float32, tag="sc",
                                         name="sc_ps", bufs=3)
                        for cc in range(4):
                            nc.tensor.matmul(sc_ps[:, :nch * 128],
                                             rxk[:, cc],
                                             ktp[:, cc, :nch * 128],
                                             start=(cc == 0), stop=False)
                        nc.tensor.matmul(sc_ps[:, :nch * 128],
                                         rxk[:DR, 4],
                                         ptp2[:, :nch * 128],
                                         start=False, stop=True)
                        attn_nat = ab.tile([128, 512], dt.bfloat16,
                                           tag="attn_nat", bufs=3)
                        if masked:
                            mrow = ab.tile([1, 512], dt.bfloat16, tag="mrow",
                                           bufs=2)
                            nc.sync.dma_start(
                                mrow[:, :nch * 128],
                                maskt[s].rearrange(
                                    "c p -> (c p)")[bsl].rearrange(
                                    "x -> 1 x"))
                            mb_ps = aps.tile([128, 512], dt.bfloat16,
                                             tag="mb", name="mb_ps", bufs=1)
                            nc.tensor.matmul(mb_ps[:, :nch * 128],
                                             ones_bf[:1, :],
                                             mrow[:, :nch * 128],
                                             start=True, stop=True)
                            mbs = ab.tile([128, 512], dt.bfloat16,
                                          tag="mbs", bufs=2)
                            nc.vector.tensor_copy(mbs[:, :nch * 128],
                                                  mb_ps[:, :nch * 128])
                            nc.scalar.activation(
                                attn_nat[:, :nch * 128],
                                sc_ps[:, :nch * 128], AF.Exp)
                            nc.vector.tensor_tensor(
                                attn_nat[:, :nch * 128],
                                attn_nat[:, :nch * 128],
                                mbs[:, :nch * 128], op=OP.mult)
                            nc.vector.tensor_reduce(
                                dent[:, blk:blk + 1],
                                attn_nat[:, :nch * 128], axis=AX.X,
                                op=OP.add)
                        else:
                            nc.scalar.activation(
                                attn_nat[:, :nch * 128],
                                sc_ps[:, :nch * 128], AF.Exp,
                                accum_out=dent[:, blk:blk + 1])
                        for j in range(nch):
                            ch = 4 * blk + j
                            atT_ps = aps.tile([128, H], dt.bfloat16,
                                              tag="ctf", name="atT_ps",
                                              bufs=2 if masked else 3)
                            nc.tensor.transpose(
                                atT_ps[:],
                                attn_nat[:, j * 128:(j + 1) * 128],
                                ident[:])
                            atT = ab.tile([128, H], dt.bfloat16, tag="atT",
                                          bufs=4)
                            nc.vector.tensor_copy(atT[:], atT_ps[:])
                            knp = ab.tile([128, KVLR], dt.bfloat16,
                                          tag="knp", bufs=4, name="knp")
                            nc.sync.dma_start(knp[:], keysN[s, ch])
                            nc.tensor.matmul(ctx_ps[:], atT[:], knp[:],
                                             start=(ch == 0),
                                             stop=(ch == lastc))
                    den4 = ab.tile([128, 1], dt.float32, tag="den4")
                    nc.vector.tensor_reduce(den4[:], dent[:, :nblk],
                                            axis=AX.X, op=OP.add)
                    inv_col = ab.tile([128, 1], dt.float32, tag="inv_col")
                    nc.vector.reciprocal(inv_col[:], den4[:])
                    if s == 0:
                        nc.sync.dma_start(war2_in[:1, :1], inv_col[:1, :1])
                        nc.gpsimd.collective_compute(
                            "AllReduce", OP.add, replica_groups=RG,
                            ins=[war2_in.opt()], outs=[war2_out.opt()])
                        nc.sync.dma_start(kvdbg[:, KVLR + 12:KVLR + 16],
                                          war2_out[:1, :4])
                    cfn = ab.tile([128, 512], dt.bfloat16, tag="cfn")
                    nc.vector.tensor_scalar_mul(cfn[:], ctx_ps[:],
                                                inv_col[:, :1])
                    for ks in range(4):
                        ctf_ps = aps.tile([128, H], dt.bfloat16, tag="ctf",
                                          name="ctf_ps",
                                          bufs=2 if masked else 3)
                        nc.tensor.transpose(
                            ctf_ps[:], cfn[:, ks * 128:(ks + 1) * 128],
                            ident[:])
                        cf = ab.tile([128, H], dt.bfloat16, tag="cf", bufs=2)
                        nc.vector.tensor_copy(cf[:], ctf_ps[:])
                        _eng = [nc.scalar, nc.sync, nc.gpsimd,
                                nc.scalar][ks]
                        _eng.dma_start(
                            a2ac_in[s][:, ks].transpose([1, 0, 2]),
                            cf[:].rearrange("p (d h) -> p d h", d=NCORES))
                    nc.gpsimd.collective_compute(
                        "AllToAll", OP.bypass, replica_groups=RG,
                        ins=[a2ac_in[s].opt()], outs=[a2ac_out[s].opt()])
              with tc.tile_pool(name="oc_ps", bufs=2, space="PSUM") as ocp:
                rxc = []
                for ks in range(4):
                    t = oc.tile([128, NCORES * SEQL * HL], dt.bfloat16,
                                tag=f"rxc{ks}", bufs=1, name=f"rxc{ks}")
                    for s_ in range(SEQL):
                        nc.scalar.dma_start(
                            t[:].rearrange("p (r s h) -> p r s h",
                                           r=NCORES, s=SEQL)[:, :, s_, :],
                            a2ac_out[s_][:, ks].transpose([1, 0, 2]))
                    rxc.append(t)
                oT = oc.tile([128, HL * B], dt.bfloat16)
                for h in range(HL):
                    op_ps = ocp.tile([128, B], dt.float32, tag="op")
                    wuvt = oc.tile([128, 4, 128], dt.bfloat16, tag="wuvt",
                                   bufs=3)
                    nc.scalar.dma_start(wuvt[:], wuv[h].transpose([1, 0, 2]))
                    for ks in range(4):
                        rhs = rxc[ks][:].rearrange(
                            "p (r s h) -> p r s h", r=NCORES,
                            s=SEQL)[:, :, :, h]
                        nc.tensor.matmul(op_ps[:], wuvt[:, ks], rhs,
                                         start=(ks == 0), stop=(ks == 3))
                    nc.vector.tensor_copy(oT[:, h * B:(h + 1) * B], op_ps[:])
                for n in range(HID // 512):
                    o_ps = ocp.tile([B, 512], dt.float32, tag="ops")
                    for h in range(HL):
                        nc.tensor.matmul(o_ps[:], oT[:, h * B:(h + 1) * B],
                                         wot_tiles[n][:, h], start=(h == 0),
                                         stop=(h == HL - 1))
                    o_sb = oc.tile([B, 512], dt.float32, tag="osb", bufs=2)
                    nc.vector.tensor_copy(o_sb[:], o_ps[:])
                    nc.scalar.dma_start(out[:, n * 512:(n + 1) * 512],
                                        o_sb[:])

    nc.compile()
    return nc


def _stage(inputs, sbn, masked, inj_by_core, n_rec, host_inject):
    """Build per-core input maps (numpy, host-side)."""
    hs = inputs["hidden_states"].astype(np.float32)
    cos = inputs["cos"].astype(np.float32)
    sin = inputs["sin"].astype(np.float32)
    Wq_a = inputs["Wq_a"].astype(np.float32)
    q_a_ln_w = inputs["q_a_ln_w"].astype(np.float32)
    Wq_b = inputs["Wq_b"].astype(np.float32)
    Wkv_a = inputs["Wkv_a"].astype(np.float32)
    kv_a_ln_w = inputs["kv_a_ln_w"].astype(np.float32)
    W_UK_T = inputs["W_UK_T"].astype(np.float32)
    W_UV = inputs["W_UV"].astype(np.float32)
    Wo = inputs["Wo"].astype(np.float32)
    cache_kv = np.asarray(inputs["cache_kv"])
    cache_pe = np.asarray(inputs["cache_pe"])
    block_table = np.asarray(inputs["block_table"])
    seq_lens = np.asarray(inputs["seq_lens"])

    ckv_flat = cache_kv.reshape(-1, KVLR)
    cpe_flat = cache_pe.reshape(-1, DR)
    if host_inject is not None:
        ckv_flat = ckv_flat.copy()
        cpe_flat = cpe_flat.copy()
        kv_lat_h, k_pe_h = host_inject
        slot_mapping = np.asarray(inputs["slot_mapping"])
        for b in range(B):
            ckv_flat[slot_mapping[b]] = kv_lat_h[b]
            cpe_flat[slot_mapping[b]] = k_pe_h[b]

    # scale + ln_w folded into Wq_b
    Wq_b_f = (Wq_b * q_a_ln_w[:, None]) * SCALE
    hiddenT = np.ascontiguousarray(hs.T).reshape(NHID, 128, B).astype(BF16)
    cosrep = np.tile(cos, (1, HL)).astype(np.float32)
    sinrep = np.tile(sin, (1, HL)).astype(np.float32)
    kvw = np.broadcast_to(kv_a_ln_w, (B, KVLR)).astype(np.float32)

    in_maps = []
    for c in range(NCORES):
        m = {}
        m["hiddenT"] = hiddenT
        wq_sh = Wq_a[:, c * QKV_SH:(c + 1) * QKV_SH]
        wkv_sh = Wkv_a[:, c * KV_SH:(c + 1) * KV_SH]
        m["wqakv"] = np.concatenate([wq_sh, wkv_sh], axis=1) \
            .reshape(NHID, 128, AKV).astype(BF16)
        hsl = slice(c * HL, (c + 1) * HL)
        m["wqb"] = Wq_b_f[:, c * HL * 192:(c + 1) * HL * 192] \
            .reshape(NQLR, 128, HL * 192).astype(BF16)
        # wuk[h][ksub][dn][k]: W_UK_T[h] is [DN, KVLR]; ksub splits k
        m["wuk"] = np.ascontiguousarray(
            W_UK_T[hsl].reshape(HL, DN, 4, 128).transpose(0, 2, 1, 3)
        ).astype(BF16)
        m["wuv"] = np.ascontiguousarray(
            W_UV[hsl].reshape(HL, 4, 128, DV)).astype(BF16)
        m["wo"] = np.ascontiguousarray(
            Wo.reshape(H, DV, HID)[hsl]).astype(BF16)

        ckvT = np.empty((SEQL, 4, 4, 128, 512), dtype=BF16)
        cpeT = np.empty((SEQL, DR, S), dtype=BF16)
        keysN = np.empty((SEQL, NBLK_PER_SEQ, 128, KVLR), dtype=BF16)
        for sl_ in range(SEQL):
            g = c * SEQL + sl_
            rows = (block_table[g][:, None] * BLK +
                    np.arange(BLK)[None, :]).reshape(-1)
            gk = ckv_flat[rows]                      # [S, KVLR]
            gp = cpe_flat[rows]                      # [S, DR]
            # [4cc,128dd,4blk,512] -> [4blk,4cc,128dd,512]
            ckvT[sl_] = gk.T.reshape(4, 128, 4, 512).transpose(
                2, 0, 1, 3).astype(BF16)
            cpeT[sl_] = gp.T.astype(BF16)
            keysN[sl_] = gk.reshape(NBLK_PER_SEQ, 128, KVLR).astype(BF16)
        m["ckvT"] = ckvT
        m["cpeT"] = cpeT
        m["keysN"] = keysN
        m["cosrep"] = cosrep
        m["sinrep"] = sinrep
        m["cospe"] = cos
        m["sinpe"] = sin
        m["kvw"] = kvw
        selm = np.zeros((B, n_rec), dtype=BF16)
        if inj_by_core is not None:
            for j, (_s, _p, b) in enumerate(inj_by_core[c]):
                selm[b, j] = 1.0
        m["sel"] = selm
        if masked:
            mk = np.zeros((SEQL, NBLK_PER_SEQ, 128), dtype=BF16)
            for sl_ in range(SEQL):
                L = int(seq_lens[c * SEQL + sl_])
                flat = mk[sl_].reshape(-1)
                flat[:L] = 1.0
            m["maskt"] = mk
        in_maps.append(m)
    return in_maps


def _analyze(inputs):
    """Derive program-shape metadata from the int inputs."""
    seq_lens = np.asarray(inputs["seq_lens"])
    block_table = np.asarray(inputs["block_table"])
    slot_mapping = np.asarray(inputs["slot_mapping"])

    sbn = []
    for sl_ in range(SEQL):
        m = 1
        for c in range(NCORES):
            L = int(seq_lens[c * SEQL + sl_])
            m = max(m, (max(L, 1) + BLK - 1) // BLK)
        sbn.append(m)
    masked = any(
        int(seq_lens[c * SEQL + sl_]) != sbn[sl_] * BLK
        for c in range(NCORES) for sl_ in range(SEQL))

    # injection sites: last write wins per (seq, pos)
    site = {}
    for b in range(B):
        slot = int(slot_mapping[b])
        blk, off = divmod(slot, BLK)
        hits = np.argwhere(block_table == blk)
        for s, j in hits:
            p = int(j) * BLK + int(off)
            if p < int(seq_lens[s]):
                site[(int(s), p)] = b
    by_core = [[] for _ in range(NCORES)]
    for (s, p), b in sorted(site.items()):
        by_core[s // SEQL].append((s % SEQL, p, b))
    pats = [tuple((s, p) for s, p, _b in lst) for lst in by_core]
    uniform = all(p == pats[0] for p in pats)
    if uniform:
        inj_records = pats[0]
        n_rec = max(1, len(inj_records))
        return tuple(sbn), masked, inj_records, [
            lst for lst in by_core], n_rec, False
    return tuple(sbn), masked, None, None, 1, True


def _host_kv(inputs):
    """Host fp32 computation of the new token's kv_lat / k_pe (fallback)."""
    hs = inputs["hidden_states"].astype(np.float32)
    kv = hs @ inputs["Wkv_a"].astype(np.float32)
    kv_lat = kv[:, :KVLR]
    var = (kv_lat * kv_lat).mean(axis=-1, keepdims=True)
    kv_lat = kv_lat / np.sqrt(var + EPS) * \
        inputs["kv_a_ln_w"].astype(np.float32)
    k_pe = kv[:, KVLR:]
    cos = inputs["cos"].astype(np.float32)
    sin = inputs["sin"].astype(np.float32)
    x1 = k_pe[:, 0::2]
    x2 = k_pe[:, 1::2]
    o = np.empty_like(k_pe)
    o[:, 0::2] = x1 * cos - x2 * sin
    o[:, 1::2] = x2 * cos + x1 * sin
    return kv_lat, o


def kernel(**inputs):
    from concourse.bass_utils import run_bass_kernel_spmd

    sbn, masked, inj_records, inj_by_core, n_rec, fallback = _analyze(inputs)
    key = (sbn, masked, inj_records, n_rec)
    if key not in _prog_cache:
        _prog_cache[key] = _build(sbn, masked, inj_records, n_rec)
    nc = _prog_cache[key]

    host_inject = _host_kv(inputs) if fallback else None
    in_maps = _stage(inputs, sbn, masked, inj_by_core, n_rec, host_inject)
    res = run_bass_kernel_spmd(nc, in_maps, core_ids=list(range(NCORES)))
    out = np.zeros((B, HID), dtype=np.float64)
    for r in res.results:
        out += r["out"].astype(np.float64)
    return out.astype(np.float32)
